# revision 27
# baseline (speedup 1.0000x reference)
"""Trainium2 Bass kernel for nn_NormDistBase (L-inf distance "matmul").

out[b, o, n] = max_d |x[b, d, n] - weight[o, d]| + bias[o]

Shapes: x [64, 1024, 49] f32, weight [1024, 1024] f32, bias [1024] f32,
out [64, 1024, 49] f32.

Strategy (8 cores = 4 batch-groups x 2 out-channel halves; per core):
  - Layout: partitions = 128 output channels (o-tile, OT=4 tiles), free =
    m=(b,n) queries (M=784), iterate d in pairs (d0, d1).
  - Per d: DMA broadcasts the x-row x[:, d, :] (fp16, from a staged DRAM
    copy) to all 128 partitions via a stride-0 source AP.
  - A-path (first A_EIGHTHS/8 of the o-tile range x m): ACT computes
    t = Abs(x_bc + (-w[:, d])) in one fused activation (the per-partition
    bias slot carries -w), DVE folds acc = max(acc, t) with a 2x fp16
    tensor_tensor.
  - D-path (rest): a custom DVE op (registered at import into
    concourse.dve_ops) computes tp = max(|xbc_d0 - w_d0|, |xbc_d1 - w_d1|)
    — two d-steps in a single 1x pass — and one 2x tensor_tensor folds tp
    into acc.  The A/D split keeps ACT (~2.3ms) and DVE (~2.5ms) balanced;
    measured 2.53 ms on 8 trn2 cores, rel err 5e-4.
  - Epilogue: ACT adds bias while upcasting fp16 -> fp32, DMA out.
"""

import os
import sys

for _p in ("/opt/trn_rl_repo",):
    if _p not in sys.path:
        sys.path.insert(0, _p)

import numpy as np

# ---- problem constants (hardcoded; kernel.py must be self-contained) ----
B, CIN, COUT, N = 64, 1024, 1024, 49
N_CORES = 8
B_SPLIT, O_SPLIT = 2, 4          # 2 batch groups x 4 out-channel quarters
B_CORE = B // B_SPLIT            # 32
O_CORE = COUT // O_SPLIT         # 256
M = B_CORE * N                   # 1568 queries per core
OT = O_CORE // 128               # 2 o-tiles per core
DC = CIN // 128                  # 8 d-chunks in the x SBUF layout

# A-path share in eighths of the 4 o-tiles (32 = all ACT, 0 = all DVE)
A_EIGHTHS = int(os.environ.get("KRN_A_EIGHTHS", "11"))
ACC_DT = os.environ.get("KRN_ACC_DT", "float16")

_PAIR_OP = None


def _register_pair_op():
    """Register the custom DVE op out = max(|in0-s0|, |in1-s1|) (one op
    covers two d-steps). Idempotent; appends to concourse.dve_ops.OPS."""
    global _PAIR_OP
    if _PAIR_OP is not None:
        return _PAIR_OP
    from concourse import dve_ops
    from concourse.dve_spec import Spec, Src0, Src1, C0, C1, AluOp, Bin, lower
    from concourse.dve_spec import _has_src1
    from concourse.dve_uop import DveOpSpec

    NAME = "PAIR_ABS_MAX_ANT"
    for op in dve_ops.OPS:
        if op.name == NAME:
            _PAIR_OP = op
            return op

    body = Bin(
        AluOp.MAX,
        Bin(AluOp.ABSOLUTE_DIFF, Src0, C0),
        Bin(AluOp.ABSOLUTE_DIFF, Src1, C1),
    )
    spec = Spec(
        body=body,
        reference=lambda in0, in1, s0, s1, imm2: np.maximum(
            np.abs(in0.astype(np.float32) - s0),
            np.abs(in1.astype(np.float32) - s1),
        ),
    )
    opcode = max(dve_ops._SUB_OPCODE_FOR_NAME.values()) + 1
    assert opcode < 0x20
    sha = {
        ver: DveOpSpec(
            name=NAME,
            opcode=opcode,
            uops=lower(spec, ver=ver),
            rd1_en=_has_src1(spec),
        ).sha(ver)
        for ver in ("v3",)
    }
    op = dve_ops.DveOp(NAME, spec, subdim=False, uops_sha=sha)
    dve_ops.OPS.append(op)
    dve_ops.CUSTOM_DVE_SPECS[NAME] = spec
    dve_ops._SUB_OPCODE_FOR_NAME[NAME] = opcode
    _PAIR_OP = op
    return op


def build(nc_d=CIN):
    import concourse.bacc as bacc
    import concourse.bass as bass
    import concourse.mybir as mybir
    from concourse.tile import TileContext
    from contextlib import ExitStack

    f32 = mybir.dt.float32
    dt16 = getattr(mybir.dt, ACC_DT)
    D = nc_d

    nc = bacc.Bacc("TRN2")
    xs = nc.dram_tensor("xs", [B_CORE, CIN, N], f32, kind="ExternalInput")
    ws = nc.dram_tensor("ws", [O_CORE, CIN], f32, kind="ExternalInput")
    bs = nc.dram_tensor("bs", [O_CORE], f32, kind="ExternalInput")
    xf16 = nc.dram_tensor("xf16", [128, DC, B_CORE, N], dt16, kind="Internal")
    out = nc.dram_tensor("out", [B_CORE, O_CORE, N], f32, kind="ExternalOutput")

    SUB = mybir.AluOpType.subtract
    MAX = mybir.AluOpType.max
    MIN = mybir.AluOpType.min
    MULT = mybir.AluOpType.mult
    AF = mybir.ActivationFunctionType
    pair_op = _register_pair_op()

    def rnd16(v):
        return (v // 16) * 16

    # per o-tile: first a_m[i] m's on the ACT path, rest on the DVE pair path
    a_m = [rnd16(min(max(A_EIGHTHS - 8 * i, 0), 8) * M // 8) for i in range(OT)]

    with ExitStack() as ctx:
        tc = ctx.enter_context(TileContext(nc))
        singles = ctx.enter_context(tc.tile_pool(name="singles", bufs=1))
        xbc_pool = ctx.enter_context(tc.tile_pool(name="xbc", bufs=6))
        t_pool = ctx.enter_context(tc.tile_pool(name="tp", bufs=6))
        out_pool = ctx.enter_context(tc.tile_pool(name="outp", bufs=2))

        xstage = singles.tile([128, DC, B_CORE, N], f32, tag="xstage")
        xsb = singles.tile([128, DC, B_CORE, N], dt16, tag="xsb")
        wsb = singles.tile([128, OT, CIN], f32, tag="wsb")
        negw = singles.tile([128, OT, CIN], f32, tag="negw")
        bsb = singles.tile([128, OT], f32, tag="bsb")
        accs = [
            singles.tile([128, M], dt16, tag=f"acc{i}", name=f"acc{i}")
            for i in range(OT)
        ]

        # loads
        xs_r = xs.ap().rearrange("b (dc p) n -> dc p b n", p=128)
        for dc in range(DC):
            nc.sync.dma_start(out=xstage[:, dc, :, :], in_=xs_r[dc])
        ws_r = ws.ap().rearrange("(ot p) d -> ot p d", p=128)
        for ot in range(OT):
            nc.sync.dma_start(out=wsb[:, ot, :], in_=ws_r[ot])
        nc.sync.dma_start(out=bsb, in_=bs.ap().rearrange("(ot p) -> p ot", p=128))
        nc.vector.tensor_copy(out=xsb, in_=xstage)  # fp32 -> fp16 once
        nc.sync.dma_start(out=xf16.ap(), in_=xsb)  # stage fp16 x to DRAM
        nc.vector.tensor_scalar(
            out=negw, in0=wsb, scalar1=-1.0, scalar2=None, op0=MULT
        )
        for a in accs:
            nc.vector.memset(a, 0.0)

        def bcast(d):
            dc, dp = d // 128, d % 128
            xbc = xbc_pool.tile([128, B_CORE, N], dt16, tag="xbc", name="xbc")
            src = xf16.ap()[dp, dc]  # [B_CORE, N] in DRAM
            src_bc = bass.AP(
                tensor=src.tensor,
                offset=src.offset,
                ap=[[0, 128]] + [list(x) for x in src.ap],
            )
            nc.sync.dma_start(out=xbc, in_=src_bc)
            return xbc.rearrange("p b n -> p (b n)")

        for d0 in range(0, D, 2):
            d1 = d0 + 1
            xb0 = bcast(d0)
            xb1 = bcast(d1) if d1 < D else None
            for i in range(OT):
                am = a_m[i]
                # ACT Abs path on m < am, for each d of the pair
                for d, xb in ((d0, xb0), (d1, xb1)):
                    if xb is None or am == 0:
                        continue
                    t = t_pool.tile([128, M], dt16, tag="t", name="t")
                    nc.scalar.activation(
                        out=t[:, 0:am],
                        in_=xb[:, 0:am],
                        func=AF.Abs,
                        bias=negw[:, i, d : d + 1],
                        scale=1.0,
                    )
                    nc.vector.tensor_tensor(
                        out=accs[i][:, 0:am],
                        in0=accs[i][:, 0:am],
                        in1=t[:, 0:am],
                        op=MAX,
                    )
                # DVE pair path on m >= am
                if am < M:
                    tp = t_pool.tile([128, M], dt16, tag="tp", name="tp")
                    if xb1 is not None:
                        nc.vector._custom_dve(
                            pair_op,
                            out=tp[:, am:M],
                            in0=xb0[:, am:M],
                            in1=xb1[:, am:M],
                            s0=wsb[:, i, d0 : d0 + 1],
                            s1=wsb[:, i, d1 : d1 + 1],
                        )
                    else:
                        raise AssertionError("D must be even")
                    nc.vector.tensor_tensor(
                        out=accs[i][:, am:M],
                        in0=accs[i][:, am:M],
                        in1=tp[:, am:M],
                        op=MAX,
                    )

        # epilogue: out = acc + bias (fp16 -> fp32), DMA to DRAM
        out_r = out.ap().rearrange("b (ot p) n -> ot p b n", p=128)
        for i in range(OT):
            o_t = out_pool.tile([128, M], f32, tag="o_t", name="o_t")
            nc.scalar.activation(
                out=o_t,
                in_=accs[i],
                func=AF.Identity,
                bias=bsb[:, i : i + 1],
                scale=1.0,
            )
            nc.sync.dma_start(
                out=out_r[i], in_=o_t.rearrange("p (b n) -> p b n", b=B_CORE)
            )

    nc.compile()
    return nc


def _shard_inputs(x, weight, bias):
    in_maps = []
    for c in range(N_CORES):
        bc, oc = c // O_SPLIT, c % O_SPLIT
        in_maps.append(
            {
                "xs": np.ascontiguousarray(x[bc * B_CORE : (bc + 1) * B_CORE]),
                "ws": np.ascontiguousarray(weight[oc * O_CORE : (oc + 1) * O_CORE]),
                "bs": np.ascontiguousarray(bias[oc * O_CORE : (oc + 1) * O_CORE]),
            }
        )
    return in_maps


def _assemble(results):
    out = np.empty((B, COUT, N), dtype=np.float32)
    for c in range(N_CORES):
        bc, oc = c // O_SPLIT, c % O_SPLIT
        out[bc * B_CORE : (bc + 1) * B_CORE, oc * O_CORE : (oc + 1) * O_CORE, :] = (
            results[c]["out"]
        )
    return out


_NC_CACHE = {}


def run(x, weight, bias, trace=False, **kw):
    from concourse import bass_utils

    key = (ACC_DT, A_EIGHTHS)
    if key not in _NC_CACHE:
        _NC_CACHE[key] = build()
    nc = _NC_CACHE[key]
    res = bass_utils.run_bass_kernel_spmd(
        nc,
        _shard_inputs(x, weight, bias),
        core_ids=list(range(N_CORES)),
        trace=trace,
        **kw,
    )
    return _assemble(res.results), res


def kernel(x, weight, bias):
    x = np.asarray(x, dtype=np.float32)
    weight = np.asarray(weight, dtype=np.float32)
    bias = np.asarray(bias, dtype=np.float32)
    out, _ = run(x, weight, bias, trace=False)
    return out


if __name__ == "__main__":
    rng = np.random.default_rng(0)
    x = rng.standard_normal((B, CIN, N), dtype=np.float32)
    w = rng.standard_normal((COUT, CIN), dtype=np.float32)
    b = np.zeros((COUT,), dtype=np.float32)
    got = kernel(x, w, b)
    exp = np.empty((B, COUT, N), np.float32)
    for bb in range(B):
        exp[bb] = np.max(np.abs(x[bb][None, :, :] - w[:, :, None]), axis=1)
    exp += b[None, :, None]
    err = np.abs(got - exp).max() / np.abs(exp).max()
    print("self-check rel err:", err)


# revision 28
# speedup vs baseline: 1.0453x; 1.0453x over previous
"""Trainium2 Bass kernel for nn_NormDistBase (L-inf distance "matmul").

out[b, o, n] = max_d |x[b, d, n] - weight[o, d]| + bias[o]

Shapes: x [64, 1024, 49] f32, weight [1024, 1024] f32, bias [1024] f32,
out [64, 1024, 49] f32.

Strategy (8 cores = 4 batch-groups x 2 out-channel halves; per core):
  - Layout: partitions = 128 output channels (o-tile, OT=4 tiles), free =
    m=(b,n) queries (M=784), iterate d in pairs (d0, d1).
  - Per d: DMA broadcasts the x-row x[:, d, :] (fp16, from a staged DRAM
    copy) to all 128 partitions via a stride-0 source AP.
  - A-path (first A_EIGHTHS/8 of the o-tile range x m): ACT computes
    t = Abs(x_bc + (-w[:, d])) in one fused activation (the per-partition
    bias slot carries -w), DVE folds acc = max(acc, t) with a 2x fp16
    tensor_tensor.
  - D-path (rest): a custom DVE op (registered at import into
    concourse.dve_ops) computes tp = max(|xbc_d0 - w_d0|, |xbc_d1 - w_d1|)
    — two d-steps in a single 1x pass — and one 2x tensor_tensor folds tp
    into acc.  The A/D split keeps ACT (~2.3ms) and DVE (~2.5ms) balanced;
    measured 2.53 ms on 8 trn2 cores, rel err 5e-4.
  - Epilogue: ACT adds bias while upcasting fp16 -> fp32, DMA out.
"""

import os
import sys

for _p in ("/opt/trn_rl_repo",):
    if _p not in sys.path:
        sys.path.insert(0, _p)

import numpy as np

# ---- problem constants (hardcoded; kernel.py must be self-contained) ----
B, CIN, COUT, N = 64, 1024, 1024, 49
N_CORES = 8
B_SPLIT, O_SPLIT = 2, 4          # 2 batch groups x 4 out-channel quarters
B_CORE = B // B_SPLIT            # 32
O_CORE = COUT // O_SPLIT         # 256
M = B_CORE * N                   # 1568 queries per core
OT = O_CORE // 128               # 2 o-tiles per core
DC = CIN // 128                  # 8 d-chunks in the x SBUF layout

# A-path share in eighths of the 4 o-tiles (32 = all ACT, 0 = all DVE)
A_EIGHTHS = int(os.environ.get("KRN_A_EIGHTHS", "11"))
ACC_DT = os.environ.get("KRN_ACC_DT", "float16")

_PAIR_OP = None


def _register_pair_op():
    """Register the custom DVE op out = max(|in0-s0|, |in1-s1|) (one op
    covers two d-steps). Idempotent; appends to concourse.dve_ops.OPS."""
    global _PAIR_OP
    if _PAIR_OP is not None:
        return _PAIR_OP
    from concourse import dve_ops
    from concourse.dve_spec import Spec, Src0, Src1, C0, C1, AluOp, Bin, lower
    from concourse.dve_spec import _has_src1
    from concourse.dve_uop import DveOpSpec

    NAME = "PAIR_ABS_MAX_ANT"
    for op in dve_ops.OPS:
        if op.name == NAME:
            _PAIR_OP = op
            return op

    body = Bin(
        AluOp.MAX,
        Bin(AluOp.ABSOLUTE_DIFF, Src0, C0),
        Bin(AluOp.ABSOLUTE_DIFF, Src1, C1),
    )
    spec = Spec(
        body=body,
        reference=lambda in0, in1, s0, s1, imm2: np.maximum(
            np.abs(in0.astype(np.float32) - s0),
            np.abs(in1.astype(np.float32) - s1),
        ),
    )
    opcode = max(dve_ops._SUB_OPCODE_FOR_NAME.values()) + 1
    assert opcode < 0x20
    sha = {
        ver: DveOpSpec(
            name=NAME,
            opcode=opcode,
            uops=lower(spec, ver=ver),
            rd1_en=_has_src1(spec),
        ).sha(ver)
        for ver in ("v3",)
    }
    op = dve_ops.DveOp(NAME, spec, subdim=False, uops_sha=sha)
    dve_ops.OPS.append(op)
    dve_ops.CUSTOM_DVE_SPECS[NAME] = spec
    dve_ops._SUB_OPCODE_FOR_NAME[NAME] = opcode
    _PAIR_OP = op
    return op


def build(nc_d=CIN):
    import concourse.bacc as bacc
    import concourse.bass as bass
    import concourse.mybir as mybir
    from concourse.tile import TileContext
    from contextlib import ExitStack

    f32 = mybir.dt.float32
    dt16 = getattr(mybir.dt, ACC_DT)
    D = nc_d

    nc = bacc.Bacc("TRN2")
    xs = nc.dram_tensor("xs", [B_CORE, CIN, N], f32, kind="ExternalInput")
    ws = nc.dram_tensor("ws", [O_CORE, CIN], f32, kind="ExternalInput")
    bs = nc.dram_tensor("bs", [O_CORE], f32, kind="ExternalInput")
    xf16s = [
        nc.dram_tensor(f"xf16_{dc}", [128, B_CORE, N], dt16, kind="Internal")
        for dc in range(DC)
    ]
    out = nc.dram_tensor("out", [B_CORE, O_CORE, N], f32, kind="ExternalOutput")

    SUB = mybir.AluOpType.subtract
    MAX = mybir.AluOpType.max
    MIN = mybir.AluOpType.min
    MULT = mybir.AluOpType.mult
    AF = mybir.ActivationFunctionType
    pair_op = _register_pair_op()

    def rnd16(v):
        return (v // 16) * 16

    # per o-tile: first a_m[i] m's on the ACT path, rest on the DVE pair path
    a_m = [rnd16(min(max(A_EIGHTHS - 8 * i, 0), 8) * M // 8) for i in range(OT)]

    with ExitStack() as ctx:
        tc = ctx.enter_context(TileContext(nc))
        singles = ctx.enter_context(tc.tile_pool(name="singles", bufs=1))
        xbc_pool = ctx.enter_context(tc.tile_pool(name="xbc", bufs=8))
        t_pool = ctx.enter_context(tc.tile_pool(name="tp", bufs=8))
        out_pool = ctx.enter_context(tc.tile_pool(name="outp", bufs=2))

        xstages = [
            singles.tile([128, B_CORE, N], f32, tag=f"xstage{dc}", name=f"xstage{dc}")
            for dc in range(DC)
        ]
        xsbs = [
            singles.tile([128, B_CORE, N], dt16, tag=f"xsb{dc}", name=f"xsb{dc}")
            for dc in range(DC)
        ]
        wsb = singles.tile([128, OT, CIN], f32, tag="wsb")
        negw = singles.tile([128, OT, CIN], f32, tag="negw")
        bsb = singles.tile([128, OT], f32, tag="bsb")
        accs = [
            singles.tile([128, M], dt16, tag=f"acc{i}", name=f"acc{i}")
            for i in range(OT)
        ]

        # loads
        xs_r = xs.ap().rearrange("b (dc p) n -> dc p b n", p=128)
        for dc in range(DC):
            nc.sync.dma_start(out=xstages[dc], in_=xs_r[dc])
            nc.vector.tensor_copy(out=xsbs[dc], in_=xstages[dc])
            nc.sync.dma_start(out=xf16s[dc].ap(), in_=xsbs[dc])
        ws_r = ws.ap().rearrange("(ot p) d -> ot p d", p=128)
        for ot in range(OT):
            nc.sync.dma_start(out=wsb[:, ot, :], in_=ws_r[ot])
        nc.sync.dma_start(out=bsb, in_=bs.ap().rearrange("(ot p) -> p ot", p=128))
        nc.vector.tensor_scalar(
            out=negw, in0=wsb, scalar1=-1.0, scalar2=None, op0=MULT
        )
        for a in accs:
            nc.vector.memset(a, 0.0)

        def bcast(d):
            dc, dp = d // 128, d % 128
            xbc = xbc_pool.tile([128, B_CORE, N], dt16, tag="xbc", name="xbc")
            src = xf16s[dc].ap()[dp]  # [B_CORE, N] in DRAM
            src_bc = bass.AP(
                tensor=src.tensor,
                offset=src.offset,
                ap=[[0, 128]] + [list(x) for x in src.ap],
            )
            nc.sync.dma_start(out=xbc, in_=src_bc)
            return xbc.rearrange("p b n -> p (b n)")

        for d0 in range(0, D, 2):
            d1 = d0 + 1
            xb0 = bcast(d0)
            xb1 = bcast(d1) if d1 < D else None
            for i in range(OT):
                am = a_m[i]
                # ACT Abs path on m < am, for each d of the pair
                for d, xb in ((d0, xb0), (d1, xb1)):
                    if xb is None or am == 0:
                        continue
                    t = t_pool.tile([128, M], dt16, tag="t", name="t")
                    nc.scalar.activation(
                        out=t[:, 0:am],
                        in_=xb[:, 0:am],
                        func=AF.Abs,
                        bias=negw[:, i, d : d + 1],
                        scale=1.0,
                    )
                    nc.vector.tensor_tensor(
                        out=accs[i][:, 0:am],
                        in0=accs[i][:, 0:am],
                        in1=t[:, 0:am],
                        op=MAX,
                    )
                # DVE pair path on m >= am
                if am < M:
                    tp = t_pool.tile([128, M], dt16, tag="tp", name="tp")
                    if xb1 is not None:
                        nc.vector._custom_dve(
                            pair_op,
                            out=tp[:, am:M],
                            in0=xb0[:, am:M],
                            in1=xb1[:, am:M],
                            s0=wsb[:, i, d0 : d0 + 1],
                            s1=wsb[:, i, d1 : d1 + 1],
                        )
                    else:
                        raise AssertionError("D must be even")
                    nc.vector.tensor_tensor(
                        out=accs[i][:, am:M],
                        in0=accs[i][:, am:M],
                        in1=tp[:, am:M],
                        op=MAX,
                    )

        # epilogue: out = acc + bias (fp16 -> fp32), DMA to DRAM
        out_r = out.ap().rearrange("b (ot p) n -> ot p b n", p=128)
        for i in range(OT):
            o_t = out_pool.tile([128, M], f32, tag="o_t", name="o_t")
            nc.scalar.activation(
                out=o_t,
                in_=accs[i],
                func=AF.Identity,
                bias=bsb[:, i : i + 1],
                scale=1.0,
            )
            nc.sync.dma_start(
                out=out_r[i], in_=o_t.rearrange("p (b n) -> p b n", b=B_CORE)
            )

    nc.compile()
    return nc


def _shard_inputs(x, weight, bias):
    in_maps = []
    for c in range(N_CORES):
        bc, oc = c // O_SPLIT, c % O_SPLIT
        in_maps.append(
            {
                "xs": np.ascontiguousarray(x[bc * B_CORE : (bc + 1) * B_CORE]),
                "ws": np.ascontiguousarray(weight[oc * O_CORE : (oc + 1) * O_CORE]),
                "bs": np.ascontiguousarray(bias[oc * O_CORE : (oc + 1) * O_CORE]),
            }
        )
    return in_maps


def _assemble(results):
    out = np.empty((B, COUT, N), dtype=np.float32)
    for c in range(N_CORES):
        bc, oc = c // O_SPLIT, c % O_SPLIT
        out[bc * B_CORE : (bc + 1) * B_CORE, oc * O_CORE : (oc + 1) * O_CORE, :] = (
            results[c]["out"]
        )
    return out


_NC_CACHE = {}


def run(x, weight, bias, trace=False, **kw):
    from concourse import bass_utils

    key = (ACC_DT, A_EIGHTHS)
    if key not in _NC_CACHE:
        _NC_CACHE[key] = build()
    nc = _NC_CACHE[key]
    res = bass_utils.run_bass_kernel_spmd(
        nc,
        _shard_inputs(x, weight, bias),
        core_ids=list(range(N_CORES)),
        trace=trace,
        **kw,
    )
    return _assemble(res.results), res


def kernel(x, weight, bias):
    x = np.asarray(x, dtype=np.float32)
    weight = np.asarray(weight, dtype=np.float32)
    bias = np.asarray(bias, dtype=np.float32)
    out, _ = run(x, weight, bias, trace=False)
    return out


if __name__ == "__main__":
    rng = np.random.default_rng(0)
    x = rng.standard_normal((B, CIN, N), dtype=np.float32)
    w = rng.standard_normal((COUT, CIN), dtype=np.float32)
    b = np.zeros((COUT,), dtype=np.float32)
    got = kernel(x, w, b)
    exp = np.empty((B, COUT, N), np.float32)
    for bb in range(B):
        exp[bb] = np.max(np.abs(x[bb][None, :, :] - w[:, :, None]), axis=1)
    exp += b[None, :, None]
    err = np.abs(got - exp).max() / np.abs(exp).max()
    print("self-check rel err:", err)
